# revision 1
# baseline (speedup 1.0000x reference)
"""Causal single-head attention on 8 Trainium2 NeuronCores.

Problem: x[4096,1024] -> Q,K,V = x@W.T+b (d_k=64), out = softmax(causal(QK^T/8)) @ V.

Strategy (sequence-parallel, uniform SPMD):
  - Query blocks of 128 rows; 32 blocks total. Core c owns global blocks
    {c, c+8, c+16, c+24} (strided) -> every core runs the IDENTICAL program.
  - Each core computes K^T/V~ for its own 512 rows, AllGathers them (split in
    two halves so the second gather overlaps band-0/1 compute), then attends
    its 4 q-blocks against the gathered keys.
  - Band schedule: band s in 0..3 attends q-slots s..3 (N = 512-128s cols)
    against shard-slot s of every rank (8 steps/band). Causality is exact:
    global kb = c'+8s vs qb = c+8j; s<j always valid, s==j masked by a
    per-core host-built mask (ones/triu/zeros by c' vs c), s>j never needed.
  - Softmax denominator comes free: V~ has a ones-column appended, so the
    AV matmul accumulates [out^T; rowsum(E)] in one pass. exp on ScalarE with
    the 1/8 scale folded in; no max-subtraction (scores are O(1) here).
  - float32r (full-rate fp32) matmuls end-to-end; all transposes of inputs
    (x^T, W^T, identity) are host-side numpy; only V^T->V~ (4) and the
    output (4) transpose on-device via PE.
"""

import os
import numpy as np
from contextlib import ExitStack

S, DM, DK = 4096, 1024, 64
NCORES = 8
QB = 128                      # rows per block
SLOTS = 4                     # q-blocks per core
SH = QB * SLOTS               # 512 shard rows per core
# per-half shard (slots 0-1 or 2-3): K^T [64, 256] + V~ [128, 2*65]
KT_H = DK * 2 * QB            # 16384
VT_H = QB * 2 * (DK + 1)      # 16640
SHARD_H = KT_H + VT_H         # 33024

USE_F32R = os.environ.get("KERNEL_F32", "0") != "1"
AMP = int(os.environ.get("KERNEL_AMP", "1"))  # repeat whole pipeline in-NEFF

LAST_EXEC_NS = None


def _build_nc():
    import concourse.bass as bass
    import concourse.bacc as bacc
    import concourse.mybir as mybir
    import concourse.tile as tile

    f32 = mybir.dt.float32
    fr = mybir.dt.float32r if USE_F32R else mybir.dt.float32
    AF = mybir.ActivationFunctionType

    nc = bacc.Bacc(None, num_devices=NCORES)

    xT_d = nc.dram_tensor("xT", [DM, SH], fr, kind="ExternalInput")
    wqkT_d = nc.dram_tensor("wqkT", [DM, 2 * DK], fr, kind="ExternalInput")
    wvT_d = nc.dram_tensor("wvT", [DM, DK], fr, kind="ExternalInput")
    bqk_d = nc.dram_tensor("bqk", [2 * DK, 1], f32, kind="ExternalInput")
    bv_d = nc.dram_tensor("bv", [DK, 1], f32, kind="ExternalInput")
    mask_d = nc.dram_tensor("mask", [NCORES * QB, SH], fr, kind="ExternalInput")
    tri_d = nc.dram_tensor("tri", [QB, QB], fr, kind="ExternalInput")
    ident_d = nc.dram_tensor("ident", [128, 128], fr, kind="ExternalInput")
    out_d = nc.dram_tensor("out", [SH, DK], f32, kind="ExternalOutput")

    with tile.TileContext(nc) as tc, ExitStack() as ctx:
        singles = ctx.enter_context(tc.tile_pool(name="singles", bufs=1))
        psum = ctx.enter_context(tc.tile_pool(name="psum", bufs=1, space="PSUM"))
        psum2 = ctx.enter_context(tc.tile_pool(name="psum2", bufs=2, space="PSUM"))
        kvpool = ctx.enter_context(tc.tile_pool(name="kvpool", bufs=3))
        epool = ctx.enter_context(tc.tile_pool(name="epool", bufs=3))
        dram = ctx.enter_context(tc.tile_pool(name="dram", bufs=1, space="DRAM"))

        # ---------------- input loads (small constants first) ----------------
        wqk_sb = singles.tile([128, DM // 128, 2 * DK], fr)
        nc.sync.dma_start(out=wqk_sb, in_=wqkT_d[:, :].rearrange("(d p) c -> p d c", p=128))
        wv_sb = singles.tile([128, DM // 128, DK], fr)
        nc.sync.dma_start(out=wv_sb, in_=wvT_d[:, :].rearrange("(d p) c -> p d c", p=128))
        bqk_sb = singles.tile([128, 1], f32)
        nc.sync.dma_start(out=bqk_sb, in_=bqk_d[:, :])
        bv_sb = singles.tile([64, 1], f32)
        nc.sync.dma_start(out=bv_sb, in_=bv_d[:, :])
        mask_sb = singles.tile([128, NCORES, SH], fr)
        tri_sb = singles.tile([128, QB], fr)
        ident_fr = singles.tile([128, 128], fr)
        nc.sync.dma_start(out=ident_fr, in_=ident_d[:, :])

        xT_sb = singles.tile([128, DM // 128, SH], fr)
        qkT_sb = singles.tile([128, SH], fr)
        vt_sb = singles.tile([128, SLOTS, DK + 1], fr)
        # ones column of V~ (f32r memset is invalid ISA; ACT writes 0*x+1)
        nc.scalar.activation(vt_sb[:, :, DK:DK + 1], ident_fr[:, 0:SLOTS].bitcast(f32),
                             AF.Identity, bias=1.0, scale=0.0)
        def load_xt_half(h):
            cs = slice(256 * h, 256 * (h + 1))
            for q in range(2):
                nc.sync.dma_start(
                    out=xT_sb[:, 4 * q:4 * (q + 1), cs],
                    in_=xT_d[512 * q:512 * (q + 1), cs].rearrange(
                        "(d p) s -> p d s", p=128))

        rep_counter = [0]

        def band_kt_ap(ag_out, s):
            t = ag_out[s // 2]
            return bass.AP(tensor=t.tensor, offset=t.offset + QB * (s % 2),
                           ap=[[2 * QB, DK], [SHARD_H, NCORES], [1, QB]])

        def band_vt_ap(ag_out, s):
            t = ag_out[s // 2]
            return bass.AP(tensor=t.tensor,
                           offset=t.offset + KT_H + (DK + 1) * (s % 2),
                           ap=[[2 * (DK + 1), QB], [SHARD_H, NCORES], [1, DK + 1]])

        def one_pass():
            # ------------- per-half: project, build V~, AllGather -------------
            r = rep_counter[0]
            rep_counter[0] += 1
            ag_in = [dram.tile([SHARD_H], fr, name=f"ag_in{r}_{h}",
                               tag=f"agi{r}_{h}") for h in range(2)]
            ag_out = [dram.tile([NCORES * SHARD_H], fr, addr_space="Shared",
                                name=f"ag_out{r}_{h}", tag=f"ago{r}_{h}")
                      for h in range(2)]
            load_xt_half(0)
            for h in range(2):
                cols = slice(256 * h, 256 * (h + 1))
                qk_ps = psum2.tile([128, 256], f32, tag="sc", bufs=3, name="qk_ps")
                v_ps = psum2.tile([64, 256], f32, tag="sc", bufs=3, name="v_ps")
                for d in range(DM // 128):
                    nc.tensor.matmul(qk_ps, lhsT=wqk_sb[:, d, :],
                                     rhs=xT_sb[:, d, cols],
                                     start=(d == 0), stop=(d == DM // 128 - 1))
                    nc.tensor.matmul(v_ps, lhsT=wv_sb[:, d, :],
                                     rhs=xT_sb[:, d, cols],
                                     start=(d == 0), stop=(d == DM // 128 - 1))
                nc.scalar.activation(qkT_sb[:, cols], qk_ps, AF.Identity,
                                     bias=bqk_sb[:, 0:1], scale=1.0)
                nc.sync.dma_start(
                    out=ag_in[h][0:KT_H].rearrange("(p s) -> p s", p=DK),
                    in_=qkT_sb[64:128, cols])
                vT_h = epool.tile([64, 256], fr, tag="vth", name="vT_h")
                nc.scalar.activation(vT_h, v_ps, AF.Identity,
                                     bias=bv_sb[:, 0:1], scale=1.0)
                for sl in range(2):
                    t_ps = psum2.tile([128, 64], fr, tag="tps", bufs=1, name="t_ps")
                    nc.tensor.transpose(t_ps, vT_h[:, 128 * sl:128 * (sl + 1)],
                                        ident_fr[0:64, 0:64])
                    nc.scalar.copy(vt_sb[:, 2 * h + sl, 0:DK], t_ps)
                nc.sync.dma_start(
                    out=ag_in[h][KT_H:SHARD_H].rearrange("(p a) -> p a", p=128),
                    in_=vt_sb[:, 2 * h:2 * (h + 1), :].rearrange("p a b -> p (a b)"))
                nc.gpsimd.collective_compute(
                    "AllGather", mybir.AluOpType.bypass,
                    replica_groups=[list(range(NCORES))],
                    ins=[ag_in[h][:]], outs=[ag_out[h][:]],
                )
                if h == 0:
                    if r == 0:
                        # 2MB of masks: behind both x^T halves; first needed
                        # only by the prepass multiplies
                        nc.sync.dma_start(
                            out=mask_sb,
                            in_=mask_d[:, :].rearrange("(c p) q -> p c q", p=128))
                        nc.sync.dma_start(out=tri_sb, in_=tri_d[:, :])
                    av_ps = psum.tile([DK + 1, SH], f32, name="av_ps")
                    for s in range(2):
                        c0, N = 128 * s, 256 - 128 * s
                        lsc = psum2.tile([128, 2, 512], f32, tag="sc", bufs=3, name="lscA")
                        le = epool.tile([128, 2, 512], fr, tag="e", name="leA")
                        lkt = kvpool.tile([DK, QB], fr, tag="lkt", name="lktA")
                        nc.sync.dma_start(
                            out=lkt,
                            in_=ag_in[0][0:KT_H].rearrange(
                                "(p s) -> p s", p=DK)[:, QB * s:QB * (s + 1)])
                        nc.tensor.matmul(lsc[:, 0, 0:N], lhsT=lkt,
                                         rhs=qkT_sb[0:64, c0:256],
                                         start=True, stop=True)
                        nc.scalar.activation(le[:, 0, 0:N], lsc[:, 0, 0:N],
                                             AF.Exp, scale=0.125)
                        nc.vector.tensor_mul(le[:, 0, 0:QB], le[:, 0, 0:QB],
                                             tri_sb)
                        nc.tensor.matmul(av_ps[:, c0:256], lhsT=vt_sb[:, s, :],
                                         rhs=le[:, 0, 0:N], start=(s == 0),
                                         stop=False, skip_group_check=True)
                    load_xt_half(1)


            # ---- local prepass part B: own blocks vs Q cols 256:512 ----
            # (part A ran inside the h-loop right after half 0; see below)
            for s in range(SLOTS):
                c0 = max(256, 128 * s)
                N = SH - c0
                lsc = psum2.tile([128, 2, 512], f32, tag="sc", bufs=3, name="lscB")
                le = epool.tile([128, 2, 512], fr, tag="e", name="leB")
                lkt = kvpool.tile([DK, QB], fr, tag="lkt", name="lktB")
                nc.sync.dma_start(
                    out=lkt,
                    in_=ag_in[s // 2][0:KT_H].rearrange(
                        "(p s) -> p s", p=DK)[:, QB * (s % 2):QB * (s % 2 + 1)])
                nc.tensor.matmul(lsc[:, 0, 0:N], lhsT=lkt,
                                 rhs=qkT_sb[0:64, c0:SH], start=True, stop=True)
                nc.scalar.activation(le[:, 0, 0:N], lsc[:, 0, 0:N], AF.Exp,
                                     scale=0.125)
                if s >= 2:   # diagonal strip lies in these columns
                    nc.vector.tensor_mul(le[:, 0, 0:QB], le[:, 0, 0:QB], tri_sb)
                nc.tensor.matmul(av_ps[:, c0:SH], lhsT=vt_sb[:, s, :],
                                 rhs=le[:, 0, 0:N], start=False, stop=False,
                                 skip_group_check=True)

            # ---------------- attention bands ----------------
            first_av = False
            for s in range(SLOTS):
                N = SH - 128 * s
                q_ap = qkT_sb[0:64, 128 * s:SH]
                ktb = kvpool.tile([DK, NCORES, QB], fr, tag="ktb", name="ktb")
                vtb = kvpool.tile([QB, NCORES, DK + 1], fr, tag="vtb", name="vtb")
                nc.sync.dma_start(out=ktb, in_=band_kt_ap(ag_out, s))
                nc.sync.dma_start(out=vtb, in_=band_vt_ap(ag_out, s))
                W = 2 if s < 2 else 4       # steps per exp; N<=256 fits 4/tile
                for g in range(NCORES // W):
                    sc_ps = psum2.tile([128, W, 1024 // W], f32, tag="sc",
                                       bufs=3, name="sc_ps")
                    e_sb = epool.tile([128, W, 1024 // W], fr, tag="e", name="e_sb")
                    for hh in range(W):
                        cp = W * g + hh
                        nc.tensor.matmul(sc_ps[:, hh, 0:N], lhsT=ktb[:, cp, :],
                                         rhs=q_ap, start=True, stop=True)
                    nc.scalar.activation(e_sb[:, :, 0:N], sc_ps[:, :, 0:N], AF.Exp,
                                         scale=0.125)
                    for hh in range(W):
                        cp = W * g + hh
                        nc.vector.tensor_mul(e_sb[:, hh, 0:N], e_sb[:, hh, 0:N],
                                             mask_sb[:, cp, 0:N])
                        last_av = (s == SLOTS - 1 and g == NCORES // W - 1
                                   and hh == W - 1)
                        nc.tensor.matmul(av_ps[:, 128 * s:SH], lhsT=vtb[:, cp, :],
                                         rhs=e_sb[:, hh, 0:N],
                                         start=first_av, stop=last_av,
                                         skip_group_check=True)
                        first_av = False

            # ------------- epilogue: transpose, normalize, store -------------
            av_sb = singles.tile([DK + 1, SH], f32, name="av_sb")
            nc.scalar.copy(av_sb, av_ps)
            out_sb = singles.tile([128, SLOTS, DK], f32, name="out_sb")
            for sl in range(SLOTS):
                t2 = psum2.tile([128, DK + 1], f32, tag="tps", bufs=1, name="t2")
                nc.tensor.transpose(t2, av_sb[0:DK + 1, 128 * sl:128 * (sl + 1)],
                                    ident_fr[0:DK + 1, 0:DK + 1].bitcast(f32))
                rec = epool.tile([128, 1], f32, tag="rec", name="rec")
                nc.vector.reciprocal(rec, t2[:, DK:DK + 1])
                nc.vector.tensor_scalar_mul(out_sb[:, sl, :], t2[:, 0:DK], rec)
                nc.sync.dma_start(out=out_d[128 * sl:128 * (sl + 1), :],
                                  in_=out_sb[:, sl, :])

        for _rep in range(AMP):
            one_pass()

    nc.finalize()
    return nc


def _in_maps(x, Wq, bq, Wk, bk, Wv, bv):
    wqkT = np.ascontiguousarray(np.concatenate([Wq.T, Wk.T], axis=1), dtype=np.float32)
    wvT = np.ascontiguousarray(Wv.T, dtype=np.float32)
    bqk = np.concatenate([bq, bk]).reshape(2 * DK, 1).astype(np.float32)
    bvv = bv.reshape(DK, 1).astype(np.float32)
    tri = np.triu(np.ones((QB, QB), dtype=np.float32))  # E^T[k,q] valid iff k<=q
    maps = []
    for c in range(NCORES):
        rows = np.concatenate([np.arange(QB * (c + 8 * sl), QB * (c + 8 * sl) + QB)
                               for sl in range(SLOTS)])
        xT = np.ascontiguousarray(x[rows].T, dtype=np.float32)  # [1024, 512]
        # [c', k, q-col] over the full 512-col band window. strip = first 128
        # cols (q-slot s); own position contributes via the local prepass.
        m = np.zeros((NCORES, QB, SH), dtype=np.float32)
        m[:c] = 1.0                   # earlier ranks: fully valid
        m[c + 1:, :, QB:] = 1.0       # later ranks: valid beyond the strip
        maps.append({
            "xT": xT, "wqkT": wqkT, "wvT": wvT, "bqk": bqk, "bv": bvv,
            "mask": np.ascontiguousarray(m.reshape(NCORES * QB, SH)),
            "tri": tri, "ident": np.eye(128, dtype=np.float32),
        })
    return maps


def kernel(**inputs):
    global LAST_EXEC_NS
    x = np.asarray(inputs["x"], dtype=np.float32)
    args = [np.asarray(inputs[k], dtype=np.float32)
            for k in ("Wq", "bq", "Wk", "bk", "Wv", "bv")]
    in_maps = _in_maps(x, args[0], args[1], args[2], args[3], args[4], args[5])

    nc = _build_nc()
    from concourse.bass_utils import run_bass_kernel_spmd
    res = run_bass_kernel_spmd(nc, in_maps, core_ids=list(range(NCORES)))
    LAST_EXEC_NS = res.exec_time_ns

    out = np.zeros((S, DK), dtype=np.float32)
    for c in range(NCORES):
        r = res.results[c]["out"]
        for sl in range(SLOTS):
            b = c + 8 * sl
            out[QB * b:QB * (b + 1)] = r[QB * sl:QB * (sl + 1)]
    return out



# revision 29
# speedup vs baseline: 2.6244x; 2.6244x over previous
"""Causal single-head attention on 8 Trainium2 NeuronCores.

Problem: x[4096,1024] -> Q,K,V = x@W.T+b (d_k=64), out = softmax(causal(QK^T/8)) @ V.

Strategy v3 (sequence-parallel over queries, NO collective):
  - Query blocks of 128 rows; 32 blocks total. Core c owns global blocks
    {c, c+8, c+16, c+24} (strided) -> every core runs the IDENTICAL program.
  - Every core streams the FULL x^T (bf16, 8.4 MB) plus its own 512 query
    rows, and projects K/V for all 4096 keys locally in 512-col chunks.
    The K/V projection + attention pipeline hides entirely behind the
    x^T DMA stream; there is no AllGather (the cost of gathering K/V
    exceeds the cost of recomputing it from the streamed x).
  - bf16 operands everywhere (PSUM accumulation fp32).
  - Band schedule: band s in 0..3 attends q-cols [128s,512) against
    k-blocks 8s..8s+7. Causality is exact: only the first 128 cols
    (diagonal strip) need masking, via a [128, 8, 128] per-core mask
    (ones / tri / zeros by block-vs-core comparison).
  - Softmax denominator comes free: V~ has a ones column appended, so the
    AV matmul accumulates [out^T; rowsum(E)] in one pass. exp on ScalarE
    with the 1/8 scale folded in; no max-subtraction (scores are O(1)).
  - Per-slot epilogue: slot s's output column block is final right after
    band s, so transpose/normalize/store for slot s overlaps band s+1.
"""

import os
import numpy as np
from contextlib import ExitStack

S, DM, DK = 4096, 1024, 64
NCORES = 8
QB = 128                      # rows per block
SLOTS = 4                     # q-blocks per core
SH = QB * SLOTS               # 512 own query rows per core
NB = S // QB                  # 32 global k-blocks
CH = 512                      # x^T streaming chunk (columns)
NCH = S // CH                 # 8 chunks

AMP = int(os.environ.get("KERNEL_AMP", "1"))  # repeat whole pipeline in-NEFF
WARMUP = int(os.environ.get("KERNEL_WARMUP", "40"))

LAST_EXEC_NS = None


def _build_nc():
    import concourse.bass as bass
    import concourse.bacc as bacc
    import concourse.mybir as mybir
    import concourse.tile as tile

    f32 = mybir.dt.float32
    bf16 = mybir.dt.bfloat16
    AF = mybir.ActivationFunctionType

    nc = bacc.Bacc(None, num_devices=NCORES)

    # xq = own 512 query rows (transposed); xk = full x^T in global order
    xq_d = nc.dram_tensor("xq", [DM, SH], bf16, kind="ExternalInput")
    xk_d = nc.dram_tensor("xk", [DM, S], bf16, kind="ExternalInput")
    # all bf16 constants packed into one tensor: [ident 128 | wkv 1024 |
    # wq 512 | mask 1024] = [128, 2688]
    cb_d = nc.dram_tensor("cb", [128, 2688], bf16, kind="ExternalInput")
    # all f32 constants packed: [bkv 1 | bq 1 | identf 65] = [128, 67]
    cf_d = nc.dram_tensor("cf", [128, 67], f32, kind="ExternalInput")
    out_d = nc.dram_tensor("out", [SH, DK], f32, kind="ExternalOutput")

    with tile.TileContext(nc) as tc, ExitStack() as ctx:
        singles = ctx.enter_context(tc.tile_pool(name="singles", bufs=1))
        psum = ctx.enter_context(tc.tile_pool(name="psum", bufs=1, space="PSUM"))
        psum2 = ctx.enter_context(tc.tile_pool(name="psum2", bufs=2, space="PSUM"))
        epool = ctx.enter_context(tc.tile_pool(name="epool", bufs=3))

        # -------- input loads, critical-path first --------
        cb_sb = singles.tile([128, 2688], bf16)
        nc.sync.dma_start(out=cb_sb[:, 0:1664], in_=cb_d[:, 0:1664])
        ident_sb = cb_sb[:, 0:128]
        wkv_sb = cb_sb[:, 128:1152].rearrange("p (d c) -> p d c", d=DM // 128)
        wq_sb = cb_sb[:, 1152:1664].rearrange("p (d c) -> p d c", d=DM // 128)
        mask_sb = cb_sb[:, 1664:2688].rearrange("p (c q) -> p c q", c=NCORES)

        xq_sb = singles.tile([128, DM // 128, SH], bf16)
        nc.sync.dma_start(out=xq_sb,
                          in_=xq_d[:, :].rearrange("(d p) s -> p d s", p=128))
        cf_sb = singles.tile([128, 67], f32)
        bkv_sb = cf_sb[:, 0:1]
        bq_sb = cf_sb[0:64, 1:2]
        identf_sb = cf_sb[0:DK + 1, 2:67]
        xk_sb = singles.tile([128, DM // 128, S], bf16)
        CHUNKS = [(CH * ch, CH) for ch in range(NCH - 1)] + \
                 [(S - CH, CH // 2), (S - CH // 2, CH // 2)]

        def load_chunk(i):
            c0x, w = CHUNKS[i]
            cs = slice(c0x, c0x + w)
            nc.sync.dma_start(
                out=xk_sb[:, :, cs],
                in_=xk_d[:, cs].rearrange("(d p) s -> p d s", p=128))

        load_chunk(0)
        # mask + f32 constants arrive behind the first x chunk
        nc.sync.dma_start(out=cf_sb, in_=cf_d[:, :])
        nc.sync.dma_start(out=cb_sb[:, 1664:2688], in_=cb_d[:, 1664:2688])
        for i in range(1, len(CHUNKS)):
            load_chunk(i)

        # warm the PE p-state ramp in the idle window before xq lands:
        # 1-col matmuls chained WAW keep the busy-run alive at ~zero cost
        warm_ps = psum2.tile([128, 1], f32, tag="tps", bufs=1, name="warm_ps")
        for _ in range(WARMUP):
            nc.tensor.matmul(warm_ps, lhsT=ident_sb, rhs=ident_sb[:, 0:1],
                             start=True, stop=True)

        qT_sb = singles.tile([64, SH], bf16)
        kvT_sb = singles.tile([128, S], bf16)
        vt_sb = singles.tile([128, NB, DK + 1], bf16)
        # ones column of V~ (ACT writes 0*x+1)
        nc.scalar.activation(vt_sb[:, :, DK:DK + 1], ident_sb[0:128, 0:NB],
                             AF.Identity, bias=1.0, scale=0.0)

        rep_counter = [0]

        def one_pass():
            rep_counter[0] += 1

            # ---- Q projection (own rows only) ----
            q_ps = psum2.tile([64, SH], f32, tag="proj", bufs=2, name="q_ps")
            for d in range(DM // 128):
                nc.tensor.matmul(q_ps, lhsT=wq_sb[:, d, :], rhs=xq_sb[:, d, :],
                                 start=(d == 0), stop=(d == DM // 128 - 1))
            nc.scalar.activation(qT_sb, q_ps, AF.Identity,
                                 bias=bq_sb[:, 0:1], scale=1.0)

            def proj_chunk(i):
                c0x, w = CHUNKS[i]
                cols = slice(c0x, c0x + w)
                kv_ps = psum2.tile([128, CH], f32, tag="proj", bufs=2,
                                   name="kv_ps")
                for d in range(DM // 128):
                    nc.tensor.matmul(kv_ps[:, 0:w], lhsT=wkv_sb[:, d, :],
                                     rhs=xk_sb[:, d, cols],
                                     start=(d == 0), stop=(d == DM // 128 - 1))
                nc.scalar.activation(kvT_sb[:, cols], kv_ps[:, 0:w], AF.Identity,
                                     bias=bkv_sb[:, 0:1], scale=1.0)
                for j in range(w // QB):             # V~ for blocks in chunk
                    b = c0x // QB + j
                    t_ps = psum2.tile([128, DK], bf16, tag="tps", bufs=1,
                                      name="t_ps")
                    nc.tensor.transpose(
                        t_ps, kvT_sb[DK:128, QB * b:QB * (b + 1)],
                        ident_sb[DK:128, DK:128])
                    nc.vector.tensor_scalar_add(vt_sb[:, b, 0:DK], t_ps, 0.0)

            av_ps = psum.tile([DK + 1, SH], f32, name="av_ps")
            out_sb = singles.tile([128, SLOTS, DK], f32, name="out_sb")
            pend = None   # (s, g, W, N, c0, e_sb)

            def flush_pend(last):
                s, g, W, N, c0, e_sb = pend
                first = s == 0 and g == 0
                # strip mask on DVE; meanwhile PE runs the mask-independent
                # bulk AV (cols QB:N) straight off the exp result
                for hh in range(W):
                    cp = W * g + hh
                    nc.vector.tensor_mul(e_sb[:, hh, 0:QB], e_sb[:, hh, 0:QB],
                                         mask_sb[:, cp, :])
                if N > QB:
                    for hh in range(W):
                        b = 8 * s + W * g + hh
                        nc.tensor.matmul(av_ps[:, c0 + QB:SH],
                                         lhsT=vt_sb[:, b, :],
                                         rhs=e_sb[:, hh, QB:N],
                                         start=first and hh == 0, stop=False,
                                         skip_group_check=True)
                # no second start=True: it would reset the av bank's
                # has_written bits and turn later accumulates into overwrites.
                # First touch of the strip region lands on cleared bits and
                # overwrites, which is the correct init.
                for hh in range(W):
                    b = 8 * s + W * g + hh
                    nc.tensor.matmul(av_ps[:, c0:c0 + QB],
                                     lhsT=vt_sb[:, b, :],
                                     rhs=e_sb[:, hh, 0:QB],
                                     start=False,
                                     stop=last and hh == W - 1,
                                     skip_group_check=True)

            def epilogue_slot(sl):
                av_sl = epool.tile([DK + 1, QB], f32, tag="avsl", bufs=2,
                                   name="av_sl")
                nc.vector.tensor_scalar_add(
                    av_sl, av_ps[:, QB * sl:QB * (sl + 1)], 0.0)
                t2 = psum2.tile([128, DK + 1], f32, tag="tps", bufs=1, name="t2")
                nc.tensor.transpose(t2, av_sl, identf_sb)
                rec = epool.tile([128, 1], f32, tag="rec", bufs=2, name="rec")
                nc.vector.reciprocal(rec, t2[:, DK:DK + 1])
                nc.vector.tensor_scalar_mul(out_sb[:, sl, :], t2[:, 0:DK], rec)
                nc.sync.dma_start(out=out_d[QB * sl:QB * (sl + 1), :],
                                  in_=out_sb[:, sl, :])

            def emit_group(s, g, W):
                nonlocal pend
                c0 = QB * s
                N = SH - c0
                sc_ps = psum2.tile([128, W, 1024 // W], f32, tag="sc",
                                   bufs=2, name="sc_ps")
                e_sb = epool.tile([128, W, 1024 // W], bf16, tag="e",
                                  name="e_sb")
                for hh in range(W):
                    b = 8 * s + W * g + hh
                    nc.tensor.matmul(sc_ps[:, hh, 0:N],
                                     lhsT=kvT_sb[0:DK, QB * b:QB * (b + 1)],
                                     rhs=qT_sb[:, c0:SH],
                                     start=True, stop=True)
                nc.scalar.activation(e_sb[:, :, 0:N], sc_ps[:, :, 0:N],
                                     AF.Exp, scale=0.125)
                if pend is not None:
                    prev_s = pend[0]
                    flush_pend(last=False)
                    if prev_s != s:
                        epilogue_slot(prev_s)
                pend = (s, g, W, N, c0, e_sb)

            # interleave projection chunks with the score groups they unblock
            for s in range(SLOTS):                    # W=2 everywhere
                proj_chunk(2 * s)
                emit_group(s, 0, 2)
                emit_group(s, 1, 2)
                proj_chunk(2 * s + 1)
                emit_group(s, 2, 2)
                if s == SLOTS - 1:
                    proj_chunk(8)
                emit_group(s, 3, 2)
            flush_pend(last=True)
            epilogue_slot(SLOTS - 1)

        for _rep in range(AMP):
            one_pass()

    nc.finalize()
    return nc


def _in_maps(x, Wq, bq, Wk, bk, Wv, bv):
    import ml_dtypes
    bf = ml_dtypes.bfloat16
    # weights repacked [p, d, c] so DMA descriptors are contiguous per row
    wkvT = np.concatenate([Wk.T, Wv.T], axis=1).reshape(DM // 128, 128, 2 * DK)
    wkvT = wkvT.transpose(1, 0, 2).reshape(128, DM // 128 * 2 * DK)
    wqT = Wq.T.reshape(DM // 128, 128, DK).transpose(1, 0, 2).reshape(128, -1)
    tri = np.triu(np.ones((QB, QB), dtype=np.float32))  # E^T[k,q]: k<=q valid
    cf = np.zeros((128, 67), dtype=np.float32)
    cf[:, 0] = np.concatenate([bk, bv])
    cf[0:64, 1] = bq
    cf[0:DK + 1, 2:67] = np.eye(DK + 1, dtype=np.float32)
    xkT = np.ascontiguousarray(x.T).astype(bf)          # [1024, 4096]
    maps = []
    for c in range(NCORES):
        rows = np.concatenate([np.arange(QB * (c + 8 * sl), QB * (c + 8 * sl) + QB)
                               for sl in range(SLOTS)])
        xqT = np.ascontiguousarray(x[rows].T).astype(bf)  # [1024, 512]
        # strip mask[k, cp, q]: block b=8s+cp vs own q-block c+8s
        m = np.zeros((128, NCORES, QB), dtype=np.float32)
        m[:, :c, :] = 1.0
        m[:, c, :] = tri
        cb = np.concatenate([np.eye(128, dtype=np.float32), wkvT, wqT,
                             m.reshape(128, NCORES * QB)], axis=1)
        maps.append({
            "xq": xqT, "xk": xkT, "cb": np.ascontiguousarray(cb).astype(bf),
            "cf": cf,
        })
    return maps


def kernel(**inputs):
    global LAST_EXEC_NS
    x = np.asarray(inputs["x"], dtype=np.float32)
    args = [np.asarray(inputs[k], dtype=np.float32)
            for k in ("Wq", "bq", "Wk", "bk", "Wv", "bv")]
    in_maps = _in_maps(x, args[0], args[1], args[2], args[3], args[4], args[5])

    nc = _build_nc()
    from concourse.bass_utils import run_bass_kernel_spmd
    res = run_bass_kernel_spmd(nc, in_maps, core_ids=list(range(NCORES)))
    LAST_EXEC_NS = res.exec_time_ns

    out = np.zeros((S, DK), dtype=np.float32)
    for c in range(NCORES):
        r = res.results[c]["out"]
        for sl in range(SLOTS):
            b = c + 8 * sl
            out[QB * b:QB * (b + 1)] = r[QB * sl:QB * (sl + 1)]
    return out


# revision 43
# speedup vs baseline: 2.8177x; 1.0737x over previous
"""Causal single-head attention on 8 Trainium2 NeuronCores.

Problem: x[4096,1024] -> Q,K,V = x@W.T+b (d_k=64), out = softmax(causal(QK^T/8)) @ V.

Strategy v3 (sequence-parallel over queries, NO collective):
  - Query blocks of 128 rows; 32 blocks total. Core c owns global blocks
    {c, c+8, c+16, c+24} (strided) -> every core runs the IDENTICAL program.
  - Every core streams the FULL x^T (bf16, 8.4 MB) plus its own 512 query
    rows, and projects K/V for all 4096 keys locally in 512-col chunks.
    The K/V projection + attention pipeline hides entirely behind the
    x^T DMA stream; there is no AllGather (the cost of gathering K/V
    exceeds the cost of recomputing it from the streamed x).
  - bf16 operands everywhere (PSUM accumulation fp32).
  - Band schedule: band s in 0..3 attends q-cols [128s,512) against
    k-blocks 8s..8s+7. Causality is exact: only the first 128 cols
    (diagonal strip) need masking, via a [128, 8, 128] per-core mask
    (ones / tri / zeros by block-vs-core comparison).
  - Softmax denominator comes free: V~ has a ones column appended, so the
    AV matmul accumulates [out^T; rowsum(E)] in one pass. exp on ScalarE
    with the 1/8 scale folded in; no max-subtraction (scores are O(1)).
  - Per-slot epilogue: slot s's output column block is final right after
    band s, so transpose/normalize/store for slot s overlaps band s+1.
"""

import os
import numpy as np
from contextlib import ExitStack

S, DM, DK = 4096, 1024, 64
NCORES = 8
QB = 128                      # rows per block
SLOTS = 4                     # q-blocks per core
SH = QB * SLOTS               # 512 own query rows per core
NB = S // QB                  # 32 global k-blocks
CH = 512                      # x^T streaming chunk (columns)
NCH = S // CH                 # 8 chunks

AMP = int(os.environ.get("KERNEL_AMP", "1"))  # repeat whole pipeline in-NEFF
WARMUP = int(os.environ.get("KERNEL_WARMUP", "40"))

LAST_EXEC_NS = None


def _build_nc():
    import concourse.bass as bass
    import concourse.bacc as bacc
    import concourse.mybir as mybir
    import concourse.tile as tile

    f32 = mybir.dt.float32
    bf16 = mybir.dt.bfloat16
    AF = mybir.ActivationFunctionType

    nc = bacc.Bacc(None, num_devices=NCORES)

    # xq = own 512 query rows (transposed); xk = full x^T in global order
    xq_d = nc.dram_tensor("xq", [DM, SH], bf16, kind="ExternalInput")
    xk_d = nc.dram_tensor("xk", [DM, S], bf16, kind="ExternalInput")
    # all bf16 constants packed into one tensor: [ident 128 | wkv 1024 |
    # wq 512 | mask 1024] = [128, 2688]
    cb_d = nc.dram_tensor("cb", [128, 2688], bf16, kind="ExternalInput")
    # all f32 constants packed: [bkv 1 | bq 1 | identf 65] = [128, 67]
    cf_d = nc.dram_tensor("cf", [128, 67], f32, kind="ExternalInput")
    out_d = nc.dram_tensor("out", [SH, DK], f32, kind="ExternalOutput")

    with tile.TileContext(nc) as tc, ExitStack() as ctx:
        singles = ctx.enter_context(tc.tile_pool(name="singles", bufs=1))
        psum = ctx.enter_context(tc.tile_pool(name="psum", bufs=1, space="PSUM"))
        psum2 = ctx.enter_context(tc.tile_pool(name="psum2", bufs=2, space="PSUM"))
        epool = ctx.enter_context(tc.tile_pool(name="epool", bufs=3))

        # -------- input loads, critical-path first --------
        cb_sb = singles.tile([128, 2688], bf16)
        nc.sync.dma_start(out=cb_sb[:, 0:1664], in_=cb_d[:, 0:1664])
        ident_sb = cb_sb[:, 0:128]
        wkv_sb = cb_sb[:, 128:1152].rearrange("p (d c) -> p d c", d=DM // 128)
        wq_sb = cb_sb[:, 1152:1664].rearrange("p (d c) -> p d c", d=DM // 128)
        mask_sb = cb_sb[:, 1664:2688].rearrange("p (c q) -> p c q", c=NCORES)

        xq_sb = singles.tile([128, DM // 128, SH], bf16)
        for h in range(2):
            hs = slice(256 * h, 256 * (h + 1))
            nc.sync.dma_start(out=xq_sb[:, :, hs],
                              in_=xq_d[:, hs].rearrange("(d p) s -> p d s", p=128))
        cf_sb = singles.tile([128, 67], f32)
        bkv_sb = cf_sb[:, 0:1]
        bq_sb = cf_sb[0:64, 1:2]
        identf_sb = cf_sb[0:DK + 1, 2:67]
        xk_sb = singles.tile([128, DM // 128, S], bf16)
        CHUNKS = [(0, 256), (256, 256), (512, 256), (768, 256)] + \
                 [(CH * ch, CH) for ch in range(2, NCH - 1)] + \
                 [(S - CH, CH // 2), (S - CH // 2, CH // 2)]

        def load_chunk(i):
            c0x, w = CHUNKS[i]
            cs = slice(c0x, c0x + w)
            nc.sync.dma_start(
                out=xk_sb[:, :, cs],
                in_=xk_d[:, cs].rearrange("(d p) s -> p d s", p=128))

        load_chunk(0)
        load_chunk(1)
        # mask + f32 constants arrive behind the first x chunk
        nc.sync.dma_start(out=cf_sb, in_=cf_d[:, :])
        nc.sync.dma_start(out=cb_sb[:, 1664:2688], in_=cb_d[:, 1664:2688])
        for i in range(2, len(CHUNKS)):
            load_chunk(i)

        # warm the PE p-state ramp in the idle window before xq lands:
        # 1-col matmuls chained WAW keep the busy-run alive at ~zero cost
        warm_ps = psum2.tile([128, 1], f32, tag="tps", bufs=1, name="warm_ps")
        for _ in range(WARMUP):
            nc.tensor.matmul(warm_ps, lhsT=ident_sb, rhs=ident_sb[:, 0:1],
                             start=True, stop=True)

        qT_sb = singles.tile([64, SH], bf16)
        kvT_sb = singles.tile([128, S], bf16)
        vt_sb = singles.tile([128, NB, DK + 1], bf16)
        # ones column of V~ (ACT writes 0*x+1)
        nc.scalar.activation(vt_sb[:, :, DK:DK + 1], ident_sb[0:128, 0:NB],
                             AF.Identity, bias=1.0, scale=0.0)

        rep_counter = [0]

        def one_pass():
            rep_counter[0] += 1

            # ---- Q projection (own rows only, by halves) ----
            for h in range(2):
                hs = slice(256 * h, 256 * (h + 1))
                q_ps = psum2.tile([64, 256], f32, tag="proj", bufs=2, name="q_ps")
                for d in range(DM // 128):
                    nc.tensor.matmul(q_ps, lhsT=wq_sb[:, d, :],
                                     rhs=xq_sb[:, d, hs],
                                     start=(d == 0), stop=(d == DM // 128 - 1))
                nc.scalar.activation(qT_sb[:, hs], q_ps, AF.Identity,
                                     bias=bq_sb[:, 0:1], scale=1.0)

            def proj_chunk(i):
                c0x, w = CHUNKS[i]
                cols = slice(c0x, c0x + w)
                kv_ps = psum2.tile([128, CH], f32, tag="proj", bufs=2,
                                   name="kv_ps")
                for d in range(DM // 128):
                    nc.tensor.matmul(kv_ps[:, 0:w], lhsT=wkv_sb[:, d, :],
                                     rhs=xk_sb[:, d, cols],
                                     start=(d == 0), stop=(d == DM // 128 - 1))
                nc.scalar.activation(kvT_sb[:, cols], kv_ps[:, 0:w], AF.Identity,
                                     bias=bkv_sb[:, 0:1], scale=1.0)
                b0 = c0x // QB
                t_ps = psum2.tile([128, CH // QB, DK], bf16, tag="tps", bufs=1,
                                  name="t_ps")
                for j in range(w // QB):             # V~ for blocks in chunk
                    b = b0 + j
                    nc.tensor.transpose(
                        t_ps[:, j, :], kvT_sb[DK:128, QB * b:QB * (b + 1)],
                        ident_sb[DK:128, DK:128])
                nc.vector.tensor_scalar_add(
                    vt_sb[:, b0:b0 + w // QB, 0:DK],
                    t_ps[:, 0:w // QB, :], 0.0)

            av_ps = psum.tile([DK + 1, SH], f32, name="av_ps")
            out_sb = singles.tile([128, SLOTS, DK], f32, name="out_sb")
            pend = []     # queue of (s, g, W, N, c0, e_sb), depth 1

            def flush_pend(last):
                s, g, W, N, c0, e_sb = pend.pop(0)
                first = s == 0 and g == 0
                # strip mask on DVE; meanwhile PE runs the mask-independent
                # bulk AV (cols QB:N) straight off the exp result
                for hh in range(W):
                    cp = W * g + hh
                    nc.vector.tensor_mul(e_sb[:, hh, 0:QB], e_sb[:, hh, 0:QB],
                                         mask_sb[:, cp, :])
                if N > QB:
                    for hh in range(W):
                        b = 8 * s + W * g + hh
                        nc.tensor.matmul(av_ps[:, c0 + QB:SH],
                                         lhsT=vt_sb[:, b, :],
                                         rhs=e_sb[:, hh, QB:N],
                                         start=first and hh == 0, stop=False,
                                         skip_group_check=True)
                # no second start=True: it would reset the av bank's
                # has_written bits and turn later accumulates into overwrites.
                # First touch of the strip region lands on cleared bits and
                # overwrites, which is the correct init.
                for hh in range(W):
                    b = 8 * s + W * g + hh
                    nc.tensor.matmul(av_ps[:, c0:c0 + QB],
                                     lhsT=vt_sb[:, b, :],
                                     rhs=e_sb[:, hh, 0:QB],
                                     start=False,
                                     stop=last and hh == W - 1,
                                     skip_group_check=True)

            def epilogue_slot(sl):
                av_sl = epool.tile([DK + 1, QB], f32, tag="avsl", bufs=2,
                                   name="av_sl")
                nc.vector.tensor_scalar_add(
                    av_sl, av_ps[:, QB * sl:QB * (sl + 1)], 0.0)
                t2 = psum2.tile([128, DK + 1], f32, tag="tps", bufs=1, name="t2")
                nc.tensor.transpose(t2, av_sl, identf_sb)
                rec = epool.tile([128, 1], f32, tag="rec", bufs=2, name="rec")
                nc.vector.reciprocal(rec, t2[:, DK:DK + 1])
                nc.vector.tensor_scalar_mul(out_sb[:, sl, :], t2[:, 0:DK], rec)
                nc.sync.dma_start(out=out_d[QB * sl:QB * (sl + 1), :],
                                  in_=out_sb[:, sl, :])

            def emit_group(s, g, W):
                nonlocal pend
                c0 = QB * s
                N = SH - c0
                sc_ps = psum2.tile([128, W, 1024 // W], f32, tag="sc",
                                   bufs=2, name="sc_ps")
                e_sb = epool.tile([128, W, 1024 // W], bf16, tag="e",
                                  name="e_sb")
                for hh in range(W):
                    b = 8 * s + W * g + hh
                    nc.tensor.matmul(sc_ps[:, hh, 0:N],
                                     lhsT=kvT_sb[0:DK, QB * b:QB * (b + 1)],
                                     rhs=qT_sb[:, c0:SH],
                                     start=True, stop=True)
                nc.scalar.activation(e_sb[:, :, 0:N], sc_ps[:, :, 0:N],
                                     AF.Exp, scale=0.125)
                if len(pend) >= 1:
                    prev_s = pend[0][0]
                    flush_pend(last=False)
                    if prev_s != s and all(p[0] != prev_s for p in pend):
                        epilogue_slot(prev_s)
                pend.append((s, g, W, N, c0, e_sb))

            # interleave projection chunks with the score groups they unblock
            for g in range(4):
                proj_chunk(g)
                emit_group(0, g, 2)
            for s in range(1, SLOTS):
                proj_chunk(2 * s + 2)
                emit_group(s, 0, 2)
                emit_group(s, 1, 2)
                proj_chunk(2 * s + 3)
                emit_group(s, 2, 2)
                if s == SLOTS - 1:
                    proj_chunk(10)
                emit_group(s, 3, 2)
            while len(pend) > 1:
                prev_s = pend[0][0]
                flush_pend(last=False)
                if all(p[0] != prev_s for p in pend):
                    epilogue_slot(prev_s)
            flush_pend(last=True)
            epilogue_slot(SLOTS - 1)

        for _rep in range(AMP):
            one_pass()

    nc.finalize()
    return nc


def _in_maps(x, Wq, bq, Wk, bk, Wv, bv):
    import ml_dtypes
    bf = ml_dtypes.bfloat16
    # weights repacked [p, d, c] so DMA descriptors are contiguous per row
    wkvT = np.concatenate([Wk.T, Wv.T], axis=1).reshape(DM // 128, 128, 2 * DK)
    wkvT = wkvT.transpose(1, 0, 2).reshape(128, DM // 128 * 2 * DK)
    wqT = Wq.T.reshape(DM // 128, 128, DK).transpose(1, 0, 2).reshape(128, -1)
    tri = np.triu(np.ones((QB, QB), dtype=np.float32))  # E^T[k,q]: k<=q valid
    cf = np.zeros((128, 67), dtype=np.float32)
    cf[:, 0] = np.concatenate([bk, bv])
    cf[0:64, 1] = bq
    cf[0:DK + 1, 2:67] = np.eye(DK + 1, dtype=np.float32)
    xkT = np.ascontiguousarray(x.T).astype(bf)          # [1024, 4096]
    maps = []
    for c in range(NCORES):
        rows = np.concatenate([np.arange(QB * (c + 8 * sl), QB * (c + 8 * sl) + QB)
                               for sl in range(SLOTS)])
        xqT = np.ascontiguousarray(x[rows].T).astype(bf)  # [1024, 512]
        # strip mask[k, cp, q]: block b=8s+cp vs own q-block c+8s
        m = np.zeros((128, NCORES, QB), dtype=np.float32)
        m[:, :c, :] = 1.0
        m[:, c, :] = tri
        cb = np.concatenate([np.eye(128, dtype=np.float32), wkvT, wqT,
                             m.reshape(128, NCORES * QB)], axis=1)
        maps.append({
            "xq": xqT, "xk": xkT, "cb": np.ascontiguousarray(cb).astype(bf),
            "cf": cf,
        })
    return maps


def kernel(**inputs):
    global LAST_EXEC_NS
    x = np.asarray(inputs["x"], dtype=np.float32)
    args = [np.asarray(inputs[k], dtype=np.float32)
            for k in ("Wq", "bq", "Wk", "bk", "Wv", "bv")]
    in_maps = _in_maps(x, args[0], args[1], args[2], args[3], args[4], args[5])

    nc = _build_nc()
    from concourse.bass_utils import run_bass_kernel_spmd
    res = run_bass_kernel_spmd(nc, in_maps, core_ids=list(range(NCORES)))
    LAST_EXEC_NS = res.exec_time_ns

    out = np.zeros((S, DK), dtype=np.float32)
    for c in range(NCORES):
        r = res.results[c]["out"]
        for sl in range(SLOTS):
            b = c + 8 * sl
            out[QB * b:QB * (b + 1)] = r[QB * sl:QB * (sl + 1)]
    return out
